# revision 8
# baseline (speedup 1.0000x reference)
"""Trainium2 Bass kernel for nn_NeuralImplicitComputationGraph.

Sequential 8192-step sampling loop on one NeuronCore (the recurrence —
rec state + last-choice feedback — is strictly sequential, so the other
7 cores cannot help; collectives have ~10us floors and cannot appear in
dynamic loops).

Strategy:
  - All matvecs as stationary-weight fp32 matmuls (true fp32, 4-pass):
    weights stream into the PE array at 128 elem/cycle via LDWEIGHTS,
    activations are the 1-column moving operand. Everything stays in
    "column layout" [128, d/128] so no transposes are needed in the
    recurrent chain.
  - x @ W_in is decomposed: pe[i]@W_in[0:256] precomputed per step
    (A_idx, streamed), pe[last_e]@W_in[256:512] precomputed into a DRAM
    table (A_e, gathered by the sampled index via dynamic-offset DMA),
    pe[last_f]@W_in[512:768]+b_in precomputed into a small SBUF table
    (A_f), and rec@W_in[768:1024] computed on the PE each step.
  - Edge logits only need a 1152-wide sliding window (mask kills the
    rest); we keep a 1280-wide window of W_edge columns resident and
    recompute it per 128-half block; invalid slots are masked via a
    host-precomputed additive stream that also folds in the Gumbel
    noise and b_edge.
  - Gumbel noise replicates jax.random.categorical's internals
    (rbg PRNG) on the CPU backend, which is where the reference runs.
"""
import os
import numpy as np

import concourse.bass as bass
import concourse.bacc as bacc
import concourse.mybir as mybir
from concourse.tile import TileContext
from concourse.bass_utils import run_bass_kernel_spmd
from concourse.bass_isa import ReduceOp

F32 = mybir.dt.float32
U32 = mybir.dt.uint32
I32 = mybir.dt.int32

L = 8192          # steps
E = 4224          # edge-choice logits
H = 1024
NF = 16
NUM_INPUT = 128
NUM_PRIOR = 1024
WIN = 1280        # resident W_edge window (10 column-blocks of 128)
NCB = WIN // 128  # 10
R1_HALVES = 1152  # halves 0..1151 use window base 0 (9 blocks of 128)
NBLK2 = 23        # blocks 9..31, window base 128*k-1024

NEG = np.float32(-3.0e38)

_GUMBEL_CACHE = "/tmp/nicg_gumbel_cache_v1.npz"


def _gumbel_tables():
    if os.path.exists(_GUMBEL_CACHE):
        d = np.load(_GUMBEL_CACHE)
        return d["gE"], d["gF"]
    import jax
    import jax.numpy as jnp
    cpu = jax.devices("cpu")[0]
    with jax.default_device(cpu):
        base = jax.random.key(42)
        ekeys = jax.random.split(jax.random.fold_in(base, 0), L)
        fkeys = jax.random.split(jax.random.fold_in(base, 1), L)
        gE = np.asarray(jax.jit(
            lambda ks: jax.lax.map(lambda k: jax.random.gumbel(k, (E,), jnp.float32), ks),
            backend="cpu")(ekeys))
        gF = np.asarray(jax.jit(
            lambda ks: jax.lax.map(lambda k: jax.random.gumbel(k, (NF,), jnp.float32), ks),
            backend="cpu")(fkeys))
    try:
        np.savez(_GUMBEL_CACHE, gE=gE, gF=gF)
    except OSError:
        pass
    return gE, gF


def _col(x):
    """[..., d] -> [..., 128, d//128] column-tile layout: d -> [d%128, d//128]."""
    return np.ascontiguousarray(np.swapaxes(x.reshape(*x.shape[:-1], -1, 128), -1, -2))


def _wblocks(W, nkt, nj):
    """[K, M] -> [128, nkt, nj, 128] stationary lhsT blocks (k-local, kt, j, m)."""
    K, M = W.shape
    assert K == nkt * 128 and M == nj * 128
    return np.ascontiguousarray(
        W.reshape(nkt, 128, nj, 128).transpose(1, 0, 2, 3)).reshape(128, nkt * nj * 128)


def _wb_of_block(k):
    return max(0, 128 * k - 1024)


def _prepare(inputs):
    pe = np.asarray(inputs["position_encodings"], np.float32)      # [8192, 256]
    W_in = np.asarray(inputs["W_in"], np.float32)                  # [1024, 1024]
    b_in = np.asarray(inputs["b_in"], np.float32)
    Ws = np.asarray(inputs["Ws"], np.float32)                      # [4, 1024, 1024]
    bs = np.asarray(inputs["bs"], np.float32)                      # [4, 1024]
    W_edge = np.asarray(inputs["W_edge"], np.float32)              # [1024, 4224]
    b_edge = np.asarray(inputs["b_edge"], np.float32)
    W_fn = np.asarray(inputs["W_fn"], np.float32)                  # [1024, 16]
    b_fn = np.asarray(inputs["b_fn"], np.float32)
    W_rec = np.asarray(inputs["W_rec"], np.float32)                # [1024, 256]
    b_rec = np.asarray(inputs["b_rec"], np.float32)
    init_rec = np.asarray(inputs["init_rec"], np.float32)

    gE, gF = _gumbel_tables()

    A_idx = pe @ W_in[0:256]                                       # [8192, 1024]
    A_e = pe[:E] @ W_in[256:512]                                   # [4224, 1024]
    A_f = pe[:NF] @ W_in[512:768] + b_in                           # [16, 1024]

    # aux stream: per step [128, 19]: [:, 0:8] A_idx col, [:, 8:18] masked
    # gumbel(+b_edge) over the step's window slots, [:, 18] gF+b_fn (even steps)
    aux = np.zeros((L, 128, 19), np.float32)
    aux[:, :, 0:8] = _col(A_idx)
    halves = np.arange(L) // 2
    wbs = np.maximum(0, 128 * (np.arange(L) // 256) - 1024)
    cols = wbs[:, None] + np.arange(WIN)[None, :]                  # [L, 1280] logical col per slot
    lo = np.maximum(0, halves - NUM_PRIOR)
    hi = NUM_INPUT + halves
    valid = (cols >= lo[:, None]) & (cols < hi[:, None])
    gwin = np.take_along_axis(gE, cols, axis=1) + b_edge[cols]
    gmask = np.where(valid, gwin, NEG).astype(np.float32)          # [L, 1280]
    aux[:, :, 8:18] = np.swapaxes(gmask.reshape(L, NCB, 128), 1, 2)
    aux[0::2, 0:NF, 18] = (gF[0::2] + b_fn[None, :]).astype(np.float32)

    prep = {
        "aux": np.ascontiguousarray(aux),
        "Ae_tab": np.ascontiguousarray(_col(A_e)),                 # [4224, 128, 8]
        "Af_tab": np.ascontiguousarray(_col(A_f).transpose(1, 0, 2).reshape(128, NF * 8)),
        "W0": _wblocks(Ws[0], 8, 8), "W1": _wblocks(Ws[1], 8, 8),
        "W2": _wblocks(Ws[2], 8, 8), "W3": _wblocks(Ws[3], 8, 8),
        "Wir": _wblocks(W_in[768:1024], 2, 8),                     # [128, 2*8*128]
        "Wrec": _wblocks(W_rec, 8, 2),                             # [128, 8*2*128]
        "Wfn": np.ascontiguousarray(
            W_fn.reshape(8, 128, 16).transpose(1, 0, 2).reshape(128, 8 * 16)),
        "Wedge": np.ascontiguousarray(W_edge),                     # [1024, 4224] natural
        "bsT": np.ascontiguousarray(_col(bs).transpose(1, 0, 2).reshape(128, 4 * 8)),
        "bR": np.ascontiguousarray(_col(b_rec[None, :])[0]),       # [128, 2]
        "rec0": np.ascontiguousarray(_col(init_rec[None, :])[0]),  # [128, 2]
        "ident": np.eye(128, dtype=np.float32),
        "prio": (256.0 - np.arange(128, dtype=np.float32)).reshape(128, 1),
    }
    return prep


def _build(n_steps):
    nc = bacc.Bacc(None)
    ET = mybir.EngineType

    aux_d = nc.dram_tensor("aux", [L, 128, 19], F32, kind="ExternalInput")
    Ae_d = nc.dram_tensor("Ae_tab", [E, 128, 8], F32, kind="ExternalInput")
    Af_d = nc.dram_tensor("Af_tab", [128, NF * 8], F32, kind="ExternalInput")
    W_d = [nc.dram_tensor(f"W{i}", [128, 8192], F32, kind="ExternalInput") for i in range(4)]
    Wir_d = nc.dram_tensor("Wir", [128, 2048], F32, kind="ExternalInput")
    Wrec_d = nc.dram_tensor("Wrec", [128, 2048], F32, kind="ExternalInput")
    Wfn_d = nc.dram_tensor("Wfn", [128, 128], F32, kind="ExternalInput")
    Wedge_d = nc.dram_tensor("Wedge", [1024, E], F32, kind="ExternalInput")
    bsT_d = nc.dram_tensor("bsT", [128, 32], F32, kind="ExternalInput")
    bR_d = nc.dram_tensor("bR", [128, 2], F32, kind="ExternalInput")
    rec0_d = nc.dram_tensor("rec0", [128, 2], F32, kind="ExternalInput")
    id_d = nc.dram_tensor("ident", [128, 128], F32, kind="ExternalInput")
    prio_d = nc.dram_tensor("prio", [128, 1], F32, kind="ExternalInput")

    eslot1_d = nc.dram_tensor("eslot1", [1, 2 * R1_HALVES], I32, kind="ExternalOutput")
    f1_d = nc.dram_tensor("f1", [1, R1_HALVES], I32, kind="ExternalOutput")
    eslot2_d = nc.dram_tensor("eslot2", [NBLK2, 256], I32, kind="ExternalOutput")
    f2_d = nc.dram_tensor("f2", [NBLK2, 128], I32, kind="ExternalOutput")
    rec_d = nc.dram_tensor("rec_out", [128, 2], F32, kind="ExternalOutput")
    hdbg_d = nc.dram_tensor("hdbg", [128, 8], F32, kind="ExternalOutput")
    eldbg_d = nc.dram_tensor("eldbg", [128, 10], F32, kind="ExternalOutput")
    h0dbg_d = nc.dram_tensor("h0dbg", [128, 8], F32, kind="ExternalOutput")

    # wait-friendly: src AP view of W_edge for window loads
    Wedge_v = Wedge_d.ap().rearrange("(kt p) c -> p kt c", p=128)

    with TileContext(nc) as tc:
        with tc.tile_pool(name="cst", bufs=1) as cst, \
             tc.tile_pool(name="sb", bufs=2) as sb, \
             tc.tile_pool(name="aux", bufs=3) as auxp, \
             tc.tile_pool(name="pp", bufs=1, space="PSUM") as pp, \
             tc.tile_pool(name="pph", bufs=1, space="PSUM") as pph, \
             tc.tile_pool(name="ppt", bufs=2, space="PSUM") as ppt, \
             tc.tile_pool(name="pps", bufs=3, space="PSUM") as pps:

            # ---- resident weights/constants ----
            Wl = []
            for i in range(4):
                t = cst.tile([128, 8, 8, 128], F32, tag=f"W{i}")
                nc.sync.dma_start(t[:].rearrange("p a b m -> p (a b m)"), W_d[i].ap())
                Wl.append(t)
            Wir = cst.tile([128, 2, 8, 128], F32)
            nc.sync.dma_start(Wir[:].rearrange("p a b m -> p (a b m)"), Wir_d.ap())
            Wrec = cst.tile([128, 8, 2, 128], F32)
            nc.sync.dma_start(Wrec[:].rearrange("p a b m -> p (a b m)"), Wrec_d.ap())
            Wfn = cst.tile([128, 8, 16], F32)
            nc.sync.dma_start(Wfn[:].rearrange("p a m -> p (a m)"), Wfn_d.ap())
            Af = cst.tile([128, NF * 8], F32)
            nc.sync.dma_start(Af[:], Af_d.ap())
            bsT = cst.tile([128, 4, 8], F32)
            nc.sync.dma_start(bsT[:].rearrange("p a b -> p (a b)"), bsT_d.ap())
            bR = cst.tile([128, 2], F32)
            nc.sync.dma_start(bR[:], bR_d.ap())
            idt = cst.tile([128, 128], F32)
            nc.sync.dma_start(idt[:], id_d.ap())
            prio = cst.tile([128, 1], F32)
            nc.sync.dma_start(prio[:], prio_d.ap())
            ones_c = cst.tile([128, 1], F32)
            nc.vector.memset(ones_c[:], 1.0)
            ones_r = cst.tile([1, 128], F32)
            nc.vector.memset(ones_r[:], 1.0)

            Wed = cst.tile([128, 8, NCB, 128], F32)   # W_edge window (stationary blocks)
            for kt in range(8):
                nc.sync.dma_start(
                    Wed[:, kt].rearrange("p cb m -> p (cb m)"),
                    Wedge_v[:, kt, 0:WIN])

            # ---- persistent state ----
            rec = cst.tile([128, 2], F32)
            nc.sync.dma_start(rec[:], rec0_d.ap())
            hdbg = cst.tile([128, 8], F32)
            eldbg = cst.tile([128, 10], F32)
            h0dbg = cst.tile([128, 8], F32)
            nc.vector.memset(hdbg[:], 0.0)
            nc.vector.memset(eldbg[:], 0.0)
            nc.vector.memset(h0dbg[:], 0.0)
            Ae_sel = cst.tile([128, 8], F32)
            nc.sync.dma_start(Ae_sel[:], Ae_d.ap()[0])
            Af_sel = cst.tile([128, 8], F32)
            nc.vector.tensor_copy(Af_sel[:], Af[:, 0:8])

            def step(i_sv, d, wb_sv, echunk, epos_sv, fchunk, fpos_sv):
                """Emit one sampling step. i_sv: step index expr; d: parity (python int);
                wb_sv: window base expr (may be python 0); e/f chunk tiles + position exprs."""
                aux = auxp.tile([128, 19], F32, tag="aux")
                nc.sync.dma_start(aux[:], aux_d.ap()[i_sv])

                # h0 = rec@Wir + A_idx + A_e + A_f(+b_in)
                h0p = pph.tile([128, 8], F32, tag="hp")
                for jj in range(8):
                    for kt in range(2):
                        nc.tensor.matmul(h0p[:, jj:jj + 1], Wir[:, kt, jj], rec[:, kt:kt + 1],
                                         start=(kt == 0), stop=(kt == 1))
                h = sb.tile([128, 8], F32, tag="h")
                nc.vector.tensor_add(h[:], h0p[:], aux[:, 0:8])
                nc.vector.tensor_add(h[:], h[:], Ae_sel[:])
                nc.vector.tensor_add(h[:], h[:], Af_sel[:])
                if os.environ.get("NICG_DEBUG"):
                    nc.vector.tensor_copy(h0dbg[:], h[:])

                for l in range(4):
                    hp = pph.tile([128, 8], F32, tag="hp")
                    for jj in range(8):
                        for kt in range(8):
                            nc.tensor.matmul(hp[:, jj:jj + 1], Wl[l][:, kt, jj], h[:, kt:kt + 1],
                                             start=(kt == 0), stop=(kt == 7))
                    hb = sb.tile([128, 8], F32, tag="hb")
                    nc.vector.tensor_add(hb[:], hp[:], bsT[:, l])
                    h = sb.tile([128, 8], F32, tag="h")
                    nc.scalar.activation(h[:], hb[:], mybir.ActivationFunctionType.Gelu)

                # edge logits over the resident window
                elp = pp.tile([128, NCB], F32, tag="el")
                for cb in range(NCB):
                    for kt in range(8):
                        nc.tensor.matmul(elp[:, cb:cb + 1], Wed[:, kt, cb], h[:, kt:kt + 1],
                                         start=(kt == 0), stop=(kt == 7))
                el2 = sb.tile([128, NCB], F32, tag="el2")
                nc.vector.tensor_add(el2[:], elp[:], aux[:, 8:18])
                if os.environ.get("NICG_DEBUG"):
                    nc.vector.tensor_copy(hdbg[:], h[:])
                    nc.vector.tensor_copy(eldbg[:], el2[:])

                # 2-level argmax -> slot = c*128 + p, all comparisons in exact fp32
                mx = sb.tile([128, 8], F32, tag="mx")
                mi = sb.tile([128, 8], U32, tag="mi")
                nc.vector.max_with_indices(mx[:], mi[:], el2[:])   # per-partition: c*
                mif = sb.tile([128, 1], F32, tag="mif")
                nc.vector.tensor_copy(mif[:], mi[:, 0:1])          # u32 -> f32
                gmax = sb.tile([128, 1], F32, tag="gmax")
                nc.gpsimd.partition_all_reduce(gmax[:], mx[:, 0:1], 128, ReduceOp.max)
                eq = sb.tile([128, 1], F32, tag="eq")
                nc.vector.tensor_tensor(eq[:], mx[:, 0:1], gmax[:], op=mybir.AluOpType.is_ge)
                score = sb.tile([128, 1], F32, tag="score")
                nc.vector.tensor_mul(score[:], eq[:], prio[:])     # 256-p at winners
                smax = sb.tile([128, 1], F32, tag="smax")
                nc.gpsimd.partition_all_reduce(smax[:], score[:], 128, ReduceOp.max)
                cmsk = sb.tile([128, 1], F32, tag="cmsk")
                nc.vector.tensor_scalar(cmsk[:], mif[:], 1.0, scalar2=None,
                                        op0=mybir.AluOpType.add)
                # keep c*+1 only on the winning partition (first max): mask by score==smax
                eq2 = sb.tile([128, 1], F32, tag="eq2")
                nc.vector.tensor_tensor(eq2[:], score[:], smax[:], op=mybir.AluOpType.is_ge)
                nc.vector.tensor_mul(cmsk[:], cmsk[:], eq2[:])
                cp1 = sb.tile([128, 1], F32, tag="cp1")
                nc.gpsimd.partition_all_reduce(cp1[:], cmsk[:], 128, ReduceOp.max)
                # slot = (cp1-1)*128 + (256-smax)
                slotf = sb.tile([1, 1], F32, tag="slotf")
                nc.vector.tensor_scalar(slotf[:], cp1[0:1, 0:1], 1.0, scalar2=128.0,
                                        op0=mybir.AluOpType.subtract,
                                        op1=mybir.AluOpType.mult)
                pstar = sb.tile([1, 1], F32, tag="pstar")
                nc.vector.tensor_scalar(pstar[:], smax[0:1, 0:1], -1.0, scalar2=256.0,
                                        op0=mybir.AluOpType.mult,
                                        op1=mybir.AluOpType.add)
                nc.vector.tensor_add(slotf[:], slotf[:], pstar[:])
                sloti = sb.tile([1, 1], I32, tag="sloti")
                nc.vector.tensor_copy(sloti[:], slotf[:])
                nc.vector.tensor_copy(echunk[0:1, bass.ds(epos_sv, 1)], sloti[:])
                slotu = sb.tile([1, 1], U32, tag="slotu")
                nc.vector.tensor_copy(slotu[:], slotf[:])
                slot_sv = nc.values_load(slotu[0:1, 0:1], engines=[ET.SP],
                                         min_val=0, max_val=WIN - 1,
                                         skip_runtime_bounds_check=True)
                nc.sync.dma_start(Ae_sel[:], Ae_d.ap()[slot_sv + wb_sv])

                # function sampling (even steps only)
                if d == 0:
                    fnp = pps.tile([16, 1], F32, tag="sm")
                    for kt in range(8):
                        nc.tensor.matmul(fnp[:], Wfn[:, kt], h[:, kt:kt + 1],
                                         start=(kt == 0), stop=(kt == 7))
                    fl2 = sb.tile([16, 1], F32, tag="fl2")
                    nc.vector.tensor_add(fl2[:], fnp[:], aux[0:16, 18:19])
                    gf = sb.tile([16, 1], F32, tag="gf")
                    nc.gpsimd.partition_all_reduce(gf[:], fl2[:], 16, ReduceOp.max)
                    eqf = sb.tile([16, 1], F32, tag="eqf")
                    nc.vector.tensor_tensor(eqf[:], fl2[:], gf[:], op=mybir.AluOpType.is_ge)
                    scf = sb.tile([16, 1], F32, tag="scf")
                    nc.vector.tensor_mul(scf[:], eqf[:], prio[0:16, 0:1])
                    smf = sb.tile([16, 1], F32, tag="smf")
                    nc.gpsimd.partition_all_reduce(smf[:], scf[:], 16, ReduceOp.max)
                    fstar = sb.tile([1, 1], F32, tag="fstar")
                    nc.vector.tensor_scalar(fstar[:], smf[0:1, 0:1], -1.0, scalar2=256.0,
                                            op0=mybir.AluOpType.mult,
                                            op1=mybir.AluOpType.add)
                    fii = sb.tile([1, 1], I32, tag="fii")
                    nc.vector.tensor_copy(fii[:], fstar[:])
                    nc.vector.tensor_copy(fchunk[0:1, bass.ds(fpos_sv, 1)], fii[:])
                    fiu = sb.tile([1, 1], U32, tag="fiu")
                    nc.vector.tensor_copy(fiu[:], fstar[:])
                    f_sv = nc.values_load(fiu[0:1, 0:1], engines=[ET.DVE],
                                          min_val=0, max_val=NF - 1,
                                          skip_runtime_bounds_check=True)
                    nc.vector.tensor_copy(Af_sel[:], Af[:, bass.ds(f_sv * 8, 8)])

                # rec update: rec = (rec + h@Wrec + bR) / norm
                rp = pps.tile([128, 2], F32, tag="sm")
                for jj in range(2):
                    for kt in range(8):
                        nc.tensor.matmul(rp[:, jj:jj + 1], Wrec[:, kt, jj], h[:, kt:kt + 1],
                                         start=(kt == 0), stop=(kt == 7))
                v = sb.tile([128, 2], F32, tag="v")
                nc.vector.tensor_add(v[:], rp[:], bR[:])
                nc.vector.tensor_add(v[:], v[:], rec[:])
                sq = sb.tile([128, 2], F32, tag="sq")
                nc.vector.tensor_mul(sq[:], v[:], v[:])
                ss = sb.tile([128, 1], F32, tag="ss")
                nc.vector.reduce_sum(ss[:], sq[:], axis=mybir.AxisListType.X)
                np_ = pps.tile([1, 1], F32, tag="sm")
                nc.tensor.matmul(np_[:], ones_c[:], ss[:], start=True, stop=True)
                ns = sb.tile([1, 1], F32, tag="ns")
                nc.scalar.activation(ns[:], np_[:], mybir.ActivationFunctionType.Sqrt)
                nc.vector.tensor_scalar(ns[:], ns[:], 1e-12, scalar2=None,
                                        op0=mybir.AluOpType.max)
                nsr = sb.tile([1, 1], F32, tag="nsr")
                nc.vector.reciprocal(nsr[:], ns[:])
                nb = pps.tile([128, 1], F32, tag="sm")
                nc.tensor.matmul(nb[:], ones_r[:], nsr[:], start=True, stop=True)
                nbs = sb.tile([128, 1], F32, tag="nbs")
                nc.vector.tensor_copy(nbs[:], nb[:])
                nc.vector.tensor_scalar(rec[:], v[:], nbs[:], scalar2=None,
                                        op0=mybir.AluOpType.mult)

            # ---- region 1: halves 0..1151 (window base 0, no reloads) ----
            ec1 = cst.tile([1, 2 * R1_HALVES], I32)
            fc1 = cst.tile([1, R1_HALVES], I32)
            r1_halves = min(R1_HALVES, (n_steps + 1) // 2)
            if r1_halves > 0:
                with tc.For_i(0, r1_halves, hint_engines=(ET.PE, ET.DVE)) as j:
                    step(j * 2, 0, 0, ec1, j * 2, fc1, j)
                    step(j * 2 + 1, 1, 0, ec1, j * 2 + 1, fc1, j)
            nc.sync.dma_start(eslot1_d.ap(), ec1[:])
            nc.sync.dma_start(f1_d.ap(), fc1[:])

            # ---- region 2: blocks k=9..31 (window base 128k-1024) ----
            if n_steps > 2 * R1_HALVES:
                ec2 = cst.tile([1, 256], I32)
                fc2 = cst.tile([1, 128], I32)
                with tc.For_i(9, 32, hint_engines=(ET.PE, ET.DVE)) as k:
                    wb_sv = k * 128 - 1024
                    for kt in range(8):
                        nc.sync.dma_start(
                            Wed[:, kt].rearrange("p cb m -> p (cb m)"),
                            Wedge_v[:, kt, bass.ds(wb_sv, WIN)])
                    with tc.For_i(0, 128, hint_engines=(ET.PE, ET.DVE)) as j:
                        i0 = k * 256 + j * 2
                        step(i0, 0, wb_sv, ec2, j * 2, fc2, j)
                        step(i0 + 1, 1, wb_sv, ec2, j * 2 + 1, fc2, j)
                    nc.sync.dma_start(eslot2_d.ap()[k - 9], ec2[:])
                    nc.sync.dma_start(f2_d.ap()[k - 9], fc2[:])

            nc.sync.dma_start(rec_d.ap(), rec[:])
            nc.sync.dma_start(hdbg_d.ap(), hdbg[:])
            nc.sync.dma_start(eldbg_d.ap(), eldbg[:])
            nc.sync.dma_start(h0dbg_d.ap(), h0dbg[:])
    nc.finalize()
    return nc


def kernel(**inputs):
    n_steps = int(os.environ.get("NICG_STEPS", L))
    prep = _prepare(inputs)
    nc = _build(n_steps)
    in_map = {
        "aux": prep["aux"], "Ae_tab": prep["Ae_tab"], "Af_tab": prep["Af_tab"],
        "W0": prep["W0"], "W1": prep["W1"], "W2": prep["W2"], "W3": prep["W3"],
        "Wir": prep["Wir"], "Wrec": prep["Wrec"], "Wfn": prep["Wfn"],
        "Wedge": prep["Wedge"], "bsT": prep["bsT"], "bR": prep["bR"],
        "rec0": prep["rec0"], "ident": prep["ident"], "prio": prep["prio"],
    }
    import time as _time
    _t0 = _time.time()
    res = run_bass_kernel_spmd(nc, [in_map], core_ids=[0],
                               trace=bool(int(os.environ.get("NICG_TRACE", "0"))))
    kernel._last_run_s = _time.time() - _t0
    out = res.results[0]
    if getattr(kernel, "_last_exec_ns", None) is None or True:
        kernel._last_exec_ns = res.exec_time_ns

    eslots = np.concatenate([out["eslot1"].reshape(-1),
                             out["eslot2"].reshape(-1)])            # [8192] window slots
    wbs = np.maximum(0, 128 * (np.arange(L) // 256) - 1024)
    e_choices = (eslots + wbs).astype(np.int32)
    f_choices = np.concatenate([out["f1"].reshape(-1),
                                out["f2"].reshape(-1)]).astype(np.int32)  # [4096]
    rec_final = out["rec_out"].T.reshape(-1).astype(np.float32)     # col -> [256]
    kernel._last_raw = out
    return e_choices, f_choices, rec_final
